# revision 7
# baseline (speedup 1.0000x reference)
"""LoRALinear kernel for Trainium2 (8 NeuronCores, SPMD data-parallel).

Computes out = x @ W.T + b + SCALE*((x@gA.T)@gB.T + (x@lA.T)@lB.T)
  x: [8, 2048, 1024] f32, W: [4096, 1024], b: [4096]
  gA/lA: [8, 1024], gB/lB: [4096, 8]  ->  out: [8, 2048, 4096] f32

Strategy (per core, one batch of x each). Host ships layout-marshaled
fp16 operands (x.T per core, W.T replicated, stacked rank-16 LoRA
factors, bias); every FLOP runs on device:
  1. Prep (fills the cold-HAM window while xT/WT DMA in): bias row
     broadcast to 128 partitions via rank-1 matmuls, then W_effT
     built in place over the WT tiles: rank-16 LoRA matmul
     (SCALE*A_cat).T @ B_catT per [128,512] chunk, added into W.T by
     DVE. No PE transposes and no ScalarE casts anywhere.
  2. Main matmul, o-half outer so compute can start after only the
     first half of W_eff has landed: per 128-row s-tile, accumulate
     psum[s, o0:o0+512] over 8 k-chunks with the xT chunk stationary
     across the 4 o-tiles of the half (LDWEIGHTS amortized 4x). PSUM
     split 4 accumulating + 4 evicting banks; DVE evicts with fused
     f32 bias add; DMA straight out.

fp16 operand rounding gives ~3e-4 absmax relative error vs the f32
reference; accumulation stays f32 in PSUM.
"""
import numpy as np
from contextlib import ExitStack

import concourse.bass as bass
import concourse.tile as tile
from concourse import bacc, mybir
from concourse.bass import ts, ds
from concourse.bass_utils import run_bass_kernel_spmd

F32 = mybir.dt.float32
F16 = mybir.dt.float16

N_CORES = 8
B, S, DIN, DOUT, R = 8, 2048, 1024, 4096, 8
SCALE = 16.0 / 8
R2 = 2 * R

P = 128            # partition tile
OTILE = 512        # matmul moving free dim (one PSUM bank of f32)
KT = DIN // P      # 8 k-tiles
OT = DOUT // OTILE # 8 o-tiles
ST = S // P        # 16 s-tiles
SLAB = 512         # xT column slab for early-start DMA
NSLAB = S // SLAB  # 4


def build_nc():
    nc = bacc.Bacc("TRN2", target_bir_lowering=False, debug=False,
                   num_devices=N_CORES)
    xT = nc.dram_tensor("xT", [DIN, S], F16, kind="ExternalInput").ap()
    WT = nc.dram_tensor("WT", [DIN, DOUT], F16, kind="ExternalInput").ap()
    bvec = nc.dram_tensor("b16", [DOUT], F16, kind="ExternalInput").ap()
    A_cat = nc.dram_tensor("A_cat", [R2, DIN], F16, kind="ExternalInput").ap()
    B_catT = nc.dram_tensor("B_catT", [R2, DOUT], F16, kind="ExternalInput").ap()
    out = nc.dram_tensor("out", [S, DOUT], F32, kind="ExternalOutput").ap()

    with tile.TileContext(nc) as tc:
        with ExitStack() as ctx:
            const = ctx.enter_context(tc.tile_pool(name="const", bufs=1))
            wet_pool = ctx.enter_context(tc.tile_pool(name="wet", bufs=1))
            xt_pool = ctx.enter_context(tc.tile_pool(name="xt", bufs=1))
            out_pool = ctx.enter_context(tc.tile_pool(name="outp", bufs=8))
            ps512 = ctx.enter_context(tc.tile_pool(name="ps512", bufs=8, space="PSUM"))

            # ---- small constants ----
            acat = const.tile([R2, DIN], F16)
            nc.sync.dma_start(acat[:], A_cat)
            bcatt = const.tile([R2, DOUT], F16)
            nc.sync.dma_start(bcatt[:], B_catT)
            brow16 = const.tile([1, DOUT], F16)
            nc.sync.dma_start(brow16[:], bvec[None, :])
            ones_col = const.tile([1, P], F16)
            nc.vector.memset(ones_col[:], 1.0)

            # ---- bulk input DMAs, early-start order ----
            # xT k-tiles land slab-by-slab so s-tile 0 is ready after 1MB
            xt = [xt_pool.tile([P, S], F16, tag=f"xt{k}", name=f"xt{k}")
                  for k in range(KT)]
            # W_effT built in place over the DMA'd W.T tiles
            wet = [wet_pool.tile([P, DOUT], F16, tag=f"wet{k}", name=f"wet{k}")
                   for k in range(KT)]

            for kt in range(KT):  # W.T columns for o-half 0 land first so
                # the LoRA adds (DVE) never gate the PE's psum rotation
                nc.sync.dma_start(wet[kt][:, ds(0, DOUT // 2)],
                                  WT[ds(kt * P, P), ds(0, DOUT // 2)])
            for kt in range(KT):
                nc.sync.dma_start(xt[kt][:, ts(0, SLAB)],
                                  xT[ds(kt * P, P), ts(0, SLAB)])
            for kt in range(KT):  # W.T columns for o-half 1
                nc.sync.dma_start(wet[kt][:, ds(DOUT // 2, DOUT // 2)],
                                  WT[ds(kt * P, P), ds(DOUT // 2, DOUT // 2)])
            for sl in range(1, NSLAB):
                for kt in range(KT):
                    nc.sync.dma_start(xt[kt][:, ts(sl, SLAB)],
                                      xT[ds(kt * P, P), ts(sl, SLAB)])

            # ---- prep on PE during DMA / cold-HAM window ----
            # HAM un-throttles (1.2 -> 2.4 GHz) only after ~3.4us of
            # full-array activity; the K=1/K=16 prep matmuls below do not
            # count, so burn the cold window on full-K=128 dummy matmuls
            warm = const.tile([P, OTILE], F16)
            nc.vector.memset(warm[:], 1.0)
            psw = ps512.tile([P, OTILE], F32, tag="ps512")
            NWARM = 18
            for i in range(NWARM):
                nc.tensor.matmul(psw[:], warm[:, ds(0, P)], warm[:],
                                 start=(i == 0), stop=(i == NWARM - 1))
            warm_sink = const.tile([P, 1], F32)
            nc.vector.tensor_copy(warm_sink[:], psw[:, ds(0, 1)])

            # bias broadcast to all 128 partitions via rank-1 fp16 matmul
            bias_sb = const.tile([P, DOUT], F32)
            for ot in range(OT):
                pb = ps512.tile([P, OTILE], F32, tag="ps512")
                nc.tensor.matmul(pb[:], ones_col[:],
                                 brow16[:, ts(ot, OTILE)],
                                 start=True, stop=True)
                nc.scalar.copy(bias_sb[:, ts(ot, OTILE)], pb[:])

            def lora_group(half, kt):
                # W_effT[k, o] += (SCALE*A_cat).T @ B_catT for 4 o-tiles
                for j in range(OT // 2):
                    ot = half * (OT // 2) + j
                    pl = ps512.tile([P, OTILE], F32, tag="ps512",
                                    name=f"pl{half}_{kt}_{j}")
                    nc.tensor.matmul(pl[:], acat[:, ts(kt, P)],
                                     bcatt[:, ts(ot, OTILE)],
                                     start=True, stop=True)
                    wchunk = wet[kt][:, ts(ot, OTILE)]
                    nc.vector.tensor_tensor(wchunk, pl[:], wchunk,
                                            mybir.AluOpType.add)

            def main_half(half, interleave_next_lora=False):
                # out[s, o-half] = x @ W_effT + bias; optionally slot the
                # next half's LoRA prep between s-tiles to keep PE dense
                for st in range(ST):
                    if interleave_next_lora and 4 <= st < 4 + KT:
                        lora_group(half + 1, st - 4)
                    pos = [ps512.tile([P, OTILE], F32, tag="ps512",
                                      name=f"pos{half}_{st}_{j}")
                           for j in range(OT // 2)]
                    for kt in range(KT):
                        xchunk = xt[kt][:, ts(st, P)]
                        for j in range(OT // 2):
                            ot = half * (OT // 2) + j
                            nc.tensor.matmul(pos[j][:], xchunk,
                                             wet[kt][:, ts(ot, OTILE)],
                                             start=(kt == 0),
                                             stop=(kt == KT - 1))
                    for j in range(OT // 2):
                        ot = half * (OT // 2) + j
                        osb = out_pool.tile([P, OTILE], F32)
                        nc.vector.tensor_tensor(osb[:], pos[j][:],
                                                bias_sb[:, ts(ot, OTILE)],
                                                mybir.AluOpType.add)
                        nc.sync.dma_start(out[ts(st, P), ts(ot, OTILE)],
                                          osb[:])

            for kt in range(KT):
                lora_group(0, kt)
            main_half(0, interleave_next_lora=True)
            main_half(1)

    nc.compile()
    return nc


_NC_CACHE = None


def _get_nc():
    global _NC_CACHE
    if _NC_CACHE is None:
        _NC_CACHE = build_nc()
    return _NC_CACHE


def make_in_maps(x, W, b, global_A, global_B, local_A, local_B):
    x = np.asarray(x, dtype=np.float32)
    W = np.asarray(W, dtype=np.float32)
    b = np.asarray(b, dtype=np.float32)
    xT = np.ascontiguousarray(x.transpose(0, 2, 1).astype(np.float16))
    WT = np.ascontiguousarray(W.T.astype(np.float16))
    A_cat = np.ascontiguousarray(
        SCALE * np.concatenate([np.asarray(global_A), np.asarray(local_A)], axis=0)
    ).astype(np.float16)
    B_catT = np.ascontiguousarray(
        np.concatenate([np.asarray(global_B).T, np.asarray(local_B).T], axis=0)
    ).astype(np.float16)
    b16 = b.astype(np.float16)
    return [
        {"xT": xT[i], "WT": WT, "b16": b16, "A_cat": A_cat, "B_catT": B_catT}
        for i in range(N_CORES)
    ]


def kernel(x, W, b, global_A, global_B, local_A, local_B):
    nc = _get_nc()
    in_maps = make_in_maps(x, W, b, global_A, global_B, local_A, local_B)
    res = run_bass_kernel_spmd(nc, in_maps, list(range(N_CORES))).results
    return np.stack([res[i]["out"] for i in range(N_CORES)], axis=0)
